# revision 18
# baseline (speedup 1.0000x reference)
"""Multi-head self-attention Trainium2 kernel (8 NeuronCores, SPMD).

Problem: B=2, S=2048, D=1024, H=16, Dk=64; torch-style Linear projections
(x @ W.T + b), custom softmax: p = exp(scores/8), attn = p / (sum(p) + 1e-8).

Sharding: 32 (batch, head) pairs over 8 cores -> core c handles batch c//4,
heads [4*(c%4), 4*(c%4)+4). Each core projects only its 256 features of
q/k/v; attention is embarrassingly parallel over (b, h).

Per-core kernel (all matmuls in fp32r: fp32 with 11 mantissa bits, ~3x the
fp32 PE throughput, ~1.2e-4 rounding error):
  - inputs (host-prepped): QT = Q[b].T [1024, 2048]; WqT/WkT/WvT [1024, 256]
    (slices of W.T); biases.
  - qT/kT [256, 2048] = (W slice) @ QT + b   (transposed-space projection;
    bias added as a per-partition scalar during the PSUM->SBUF copy)
  - v     [2048, 256] = QT.T @ WvT           (normal layout; bias folded into
    the final normalize: (p@v)/denom + bv, exact because sum_t p*bv = denom*bv)
  - per head pair: scoresT[t, s] for both heads packed into one PE pass via
    tile_position row groups (0,0)/(64,0), written into one 2-bank PSUM tile
    so a single exp instruction [128,1024] covers both heads (ScalarE is the
    bottleneck engine; its fixed ~0.5us/instruction overhead is halved)
  - ctxT_ext [65, 512-chunk] = [v_h | 1].T @ p accumulated over 16 t-tiles;
    row 64 = softmax denominator
  - finalize: PE-transpose 128-col blocks -> [128, 65]; DVE reciprocal of
    col 64 and scalar_tensor_tensor: out = ctx * (1/denom) + bv

Scheduling: the attention phase is ACT(exp)-bound (~1.3us per t-step); the
PE's spare capacity there is filled with useful work -- the pair-1
projections and the transpose/normalize pipeline -- one or two units per
t-step. This both hides that work entirely and keeps the PE busy enough
that the HAM clock gate never re-throttles it to 1.2GHz (a >3.4us PE idle
anywhere would double every subsequent matmul's duration; observed).

Output per core: [2048, 256] fp32 -> host concatenates features per batch.
"""

import sys

sys.path.insert(0, "/opt/trn_rl_repo")

from collections import deque
from contextlib import ExitStack

import numpy as np

import concourse.bass as bass
import concourse.tile as tile
from concourse import bacc, mybir
from concourse.bass_utils import run_bass_kernel_spmd
from concourse.masks import make_identity

F32 = mybir.dt.float32
F32R = mybir.dt.float32r

S = 2048  # sequence length
D = 1024  # d_model
J = 256  # features per core (4 heads x 64)
NKT = 8  # k-tiles of the d_model contraction
NSC = 4  # s-chunks of 512
NTT = 16  # t-tiles of 128
N_CORES = 8

_cached_nc = None
last_result = None  # BassKernelResults of the most recent run (for test.py)


def _round_fp32r(x: np.ndarray) -> np.ndarray:
    """Round fp32 to fp32r (keep 11 mantissa bits, round to nearest even)."""
    u = np.ascontiguousarray(x, dtype=np.float32).view(np.uint32)
    r = (u.astype(np.uint64) + 0x7FF + ((u >> 12) & 1)) & 0xFFFFF000
    return r.astype(np.uint32).view(np.float32)


def _build():
    nc = bacc.Bacc(None, target_bir_lowering=False)

    qt = nc.dram_tensor("qt", [D, S], F32R, kind="ExternalInput")
    wq = nc.dram_tensor("wq", [D, J], F32R, kind="ExternalInput")
    wk = nc.dram_tensor("wk", [D, J], F32R, kind="ExternalInput")
    wv = nc.dram_tensor("wv", [D, J], F32R, kind="ExternalInput")
    bq = nc.dram_tensor("bq", [J], F32, kind="ExternalInput")
    bk = nc.dram_tensor("bk", [J], F32, kind="ExternalInput")
    bv = nc.dram_tensor("bv", [J], F32, kind="ExternalInput")
    out = nc.dram_tensor("out", [S, J], F32, kind="ExternalOutput")

    with tile.TileContext(nc) as tc, ExitStack() as ctx:
        wts = ctx.enter_context(tc.tile_pool(name="wts", bufs=1))
        qkp = ctx.enter_context(tc.tile_pool(name="qkp", bufs=1))
        vxp = ctx.enter_context(tc.tile_pool(name="vxp", bufs=1))
        bp = ctx.enter_context(tc.tile_pool(name="bp", bufs=1))
        cxp = ctx.enter_context(tc.tile_pool(name="cxp", bufs=8))
        pTp = ctx.enter_context(tc.tile_pool(name="pTp", bufs=4))
        outp = ctx.enter_context(tc.tile_pool(name="outp", bufs=1))
        rp = ctx.enter_context(tc.tile_pool(name="rp", bufs=8))
        qtcp = ctx.enter_context(tc.tile_pool(name="qtc", bufs=3))

        # Weights: 8 k-tiles each of [128, 256], k-major and split across the
        # HWDGE (sync) / SWDGE (gpsimd) queues, interleaved with the first
        # s-chunk of QT below so the k=0 projection matmuls start early
        wq_t = [
            wts.tile([128, J], F32R, name=f"wq{k}", tag=f"wq{k}") for k in range(NKT)
        ]
        wk_t = [
            wts.tile([128, J], F32R, name=f"wk{k}", tag=f"wk{k}") for k in range(NKT)
        ]
        wv_t = [
            wts.tile([128, J], F32R, name=f"wv{k}", tag=f"wv{k}") for k in range(NKT)
        ]
        qtc0 = qtcp.tile([128, NKT, 512], F32R, name="qtc0", tag="qtc")
        for k in range(NKT):
            ksl = slice(k * 128, (k + 1) * 128)
            nc.sync.dma_start(qtc0[:, k, :], qt[ksl, 0:512])
            nc.sync.dma_start(wq_t[k][:], wq[ksl, :])
            nc.sync.dma_start(wk_t[k][:], wk[ksl, :])
            nc.gpsimd.dma_start(wv_t[k][:], wv[ksl, :])

        # Biases: bq/bk as per-partition scalars [128, 2]; bv broadcast [128, 256]
        bq_t = bp.tile([128, 2], F32, name="bqt")
        nc.sync.dma_start(bq_t[:], bq.rearrange("(m p) -> p m", p=128))
        bk_t = bp.tile([128, 2], F32, name="bkt")
        nc.sync.dma_start(bk_t[:], bk.rearrange("(m p) -> p m", p=128))
        bv_t = bp.tile([128, J], F32, name="bvt")
        bvap = bv[:]
        bv_bcast = bass.AP(
            tensor=bvap.tensor, offset=bvap.offset, ap=[[0, 128], [1, J]]
        )
        nc.sync.dma_start(bv_t[:], bv_bcast)

        ident = bp.tile([128, 128], F32, name="ident")
        make_identity(nc, ident[:])
        scratch = bp.tile([128, 1], F32, name="scratch")

        # Persistent projected tensors
        qT = [qkp.tile([128, S], F32R, name=f"qT{m}", tag=f"qT{m}") for m in range(2)]
        kT = [qkp.tile([128, S], F32R, name=f"kT{m}", tag=f"kT{m}") for m in range(2)]
        v_ext = []
        for t in range(NTT):
            vt = vxp.tile([128, 4, 65], F32R, name=f"vx{t}", tag=f"vx{t}")
            nc.gpsimd.memset(vt[:].bitcast(F32), 1.0)  # ones col [:, h, 64] survives
            v_ext.append(vt)
        # out accumulation tiles, one per 128-row block of the output
        out_tiles = [
            outp.tile([128, J], F32, name=f"ot{b}", tag=f"ot{b}") for b in range(16)
        ]

        def dma_qtc(tile_, sc):
            s0 = sc * 512
            for k in range(NKT):
                nc.sync.dma_start(
                    tile_[:, k, :], qt[k * 128 : (k + 1) * 128, s0 : s0 + 512]
                )

        # ---- Phase 1: kT[0], qT[0] chunk 0, and all of v ----
        phase1_qtc = []
        with tc.tile_pool(name="pps", bufs=1, space="PSUM") as pps:
            for sc in range(NSC):
                s0 = sc * 512
                if sc == 0:
                    qtc = qtc0
                else:
                    qtc = qtcp.tile([128, NKT, 512], F32R, name="qtc", tag="qtc")
                    dma_qtc(qtc, sc)
                # qT[0] is only needed chunk-by-chunk as the pair-0 attention
                # blocks consume it, so chunks 1-3 move to attention filler
                pq = pps.tile([128, 512], F32, name="pq", tag="pq") if sc == 0 else None
                pk = pps.tile([128, 512], F32, name="pk", tag="pk")
                pv = [
                    pps.tile([128, J], F32, name=f"pv{i}", tag=f"pv{i}")
                    for i in range(4)
                ]
                for k in range(NKT):
                    st, sp = (k == 0), (k == NKT - 1)
                    if pq is not None:
                        nc.tensor.matmul(
                            pq[:], wq_t[k][:, 0:128], qtc[:, k, :], start=st, stop=sp
                        )
                    nc.tensor.matmul(
                        pk[:], wk_t[k][:, 0:128], qtc[:, k, :], start=st, stop=sp
                    )
                    for i in range(4):
                        nc.tensor.matmul(
                            pv[i][:],
                            qtc[:, k, i * 128 : (i + 1) * 128],
                            wv_t[k][:],
                            start=st,
                            stop=sp,
                        )
                if pq is not None:
                    nc.vector.tensor_scalar_add(
                        qT[0][:, s0 : s0 + 512], pq[:], bq_t[:, 0:1]
                    )
                nc.vector.tensor_scalar_add(
                    kT[0][:, s0 : s0 + 512], pk[:], bk_t[:, 0:1]
                )
                phase1_qtc.append(qtc)
                for i in range(4):
                    nc.vector.tensor_copy(
                        v_ext[sc * 4 + i][:, :, 0:64],
                        pv[i][:].rearrange("p (h d) -> p h d", h=4),
                    )
                if sc == 0:
                    # pre-load the ACT exp table set during projections so the
                    # first attention exp doesn't stall the pipeline ~2.7us
                    nc.scalar.activation(
                        scratch[:], bq_t[:, 0:1],
                        mybir.ActivationFunctionType.Exp, scale=0.0,
                    )

        # ---- Phase 2: attention, with pair-1 projections and the
        #      transpose/normalize pipeline as PE filler work ----
        with (
            tc.tile_pool(name="aps", bufs=1, space="PSUM") as aps,
            tc.tile_pool(name="p1b", bufs=1, space="PSUM") as p1b,
        ):
            # --- filler: qT[0] chunks 1-3 (read the still-resident phase-1
            #     qtc tiles; must run before proj1b recycles those slots) ---
            q0_state = {}

            def uq0_start(c):
                def f():
                    q0_state[c] = p1b.tile(
                        [128, 512], F32, name="pq0f", tag=f"x{c % 2}"
                    )
                return f

            def uq0_k(c, k):
                def f():
                    st, sp = (k == 0), (k == NKT - 1)
                    nc.tensor.matmul(
                        q0_state[c][:],
                        wq_t[k][:, 0:128],
                        phase1_qtc[c][:, k, :],
                        start=st,
                        stop=sp,
                    )
                return f

            def uq0_copy(c):
                def f():
                    s0 = c * 512
                    nc.vector.tensor_scalar_add(
                        qT[0][:, s0 : s0 + 512], q0_state.pop(c)[:], bq_t[:, 0:1]
                    )
                return f

            # --- filler: pair-1 projection work units ---
            p1_state = {}

            def u_dma(c):
                def f():
                    qtc2 = qtcp.tile([128, NKT, 512], F32R, name="qtc2", tag="qtc")
                    dma_qtc(qtc2, c)
                    px0 = p1b.tile([128, 512], F32, name="px0", tag="x0")
                    px1 = p1b.tile([128, 512], F32, name="px1", tag="x1")
                    p1_state[c] = (qtc2, px0, px1)
                return f

            def u_k(c, k):
                def f():
                    qtc2, px0, px1 = p1_state[c]
                    st, sp = (k == 0), (k == NKT - 1)
                    nc.tensor.matmul(
                        px0[:], wq_t[k][:, 128:256], qtc2[:, k, :], start=st, stop=sp
                    )
                    nc.tensor.matmul(
                        px1[:], wk_t[k][:, 128:256], qtc2[:, k, :], start=st, stop=sp
                    )
                return f

            def u_copy(c):
                def f():
                    _, px0, px1 = p1_state.pop(c)
                    s0 = c * 512
                    nc.vector.tensor_scalar_add(
                        qT[1][:, s0 : s0 + 512], px0[:], bq_t[:, 1:2]
                    )
                    nc.vector.tensor_scalar_add(
                        kT[1][:, s0 : s0 + 512], px1[:], bk_t[:, 1:2]
                    )
                return f

            work = deque()
            for c in range(1, NSC):
                work.append(uq0_start(c))
                for k in range(0, NKT, 2):
                    work.append(uq0_k(c, k))
                    work.append(uq0_k(c, k + 1))
                work.append(uq0_copy(c))
            for c in range(NSC):
                work.append(u_dma(c))
                for k in range(NKT):
                    work.append(u_k(c, k))
                work.append(u_copy(c))

            # --- filler: transpose/normalize pieces ---
            pieces = deque()
            done_cnt = {}
            piece_idx = [0]

            def piece(cs_tile, sc, h, i):
                def f():
                    tagidx = piece_idx[0] % 2
                    piece_idx[0] += 1
                    tp = p1b.tile(
                        [128, 65], F32, name="tp", tag=f"x{tagidx}"
                    )
                    nc.tensor.transpose(
                        tp[:],
                        cs_tile[0:65, i * 128 : (i + 1) * 128],
                        ident[0:65, 0:65],
                    )
                    r = rp.tile([128, 1], F32, name="r", tag="r")
                    nc.vector.reciprocal(r[:], tp[:, 64:65])
                    blk = sc * 4 + i
                    nc.vector.scalar_tensor_tensor(
                        out=out_tiles[blk][:, h * 64 : (h + 1) * 64],
                        in0=tp[:, 0:64],
                        scalar=r[:],
                        in1=bv_t[:, h * 64 : (h + 1) * 64],
                        op0=mybir.AluOpType.mult,
                        op1=mybir.AluOpType.add,
                    )
                    done_cnt[blk] = done_cnt.get(blk, 0) + 1
                    if done_cnt[blk] == 4:
                        nc.sync.dma_start(
                            out[blk * 128 : (blk + 1) * 128, :], out_tiles[blk][:]
                        )
                return f

            def fill_slot():
                # pair-1 projections first (they gate the pair-1 attention
                # blocks), then transpose pieces, which reuse the x0/x1 PSUM
                # banks after the projections retire
                if work:
                    work.popleft()()
                elif pieces:
                    pieces.popleft()()
                    if len(pieces) > 12 and pieces:
                        pieces.popleft()()

            # burst: pair-1 chunk 0 bridges the PSUM pool-transition wait so
            # the PE never idles across the phase boundary (HAM)
            for _ in range(10):
                if work:
                    work.popleft()()

            for pair in range(2):
                for sc in range(NSC):
                    s0 = sc * 512
                    hA, hB = 2 * pair, 2 * pair + 1
                    qTt, kTt = qT[pair], kT[pair]
                    ctxA = aps.tile([65, 512], F32, name="ctxA", tag="ctx", bufs=2)
                    ctxB = aps.tile([65, 512], F32, name="ctxB", tag="ctx", bufs=2)
                    pts = {}
                    for t in range(NTT + 1):
                        if t < NTT:
                            tsl = slice(t * 128, (t + 1) * 128)
                            # both heads' scoresT share one 2-bank tile so a
                            # single exp instruction covers them
                            g = aps.tile(
                                [128, 1024], F32, name="g", tag="grp", bufs=2
                            )
                            nc.tensor.matmul(
                                g[:, 0:512],
                                kTt[0:64, tsl],
                                qTt[0:64, s0 : s0 + 512],
                                start=True,
                                stop=True,
                                tile_position=(0, 0),
                            )
                            nc.tensor.matmul(
                                g[:, 512:1024],
                                kTt[64:128, tsl],
                                qTt[64:128, s0 : s0 + 512],
                                start=True,
                                stop=True,
                                tile_position=(64, 0),
                            )
                            pT_ = pTp.tile([128, 1024], F32R, name="pT_", tag="pT")
                            nc.scalar.activation(
                                pT_[:], g[:],
                                mybir.ActivationFunctionType.Exp, scale=0.125,
                            )
                            pts[t] = pT_
                        if t >= 1:
                            pT_ = pts.pop(t - 1)
                            st, sp = (t - 1 == 0), (t - 1 == NTT - 1)
                            nc.tensor.matmul(
                                ctxA[:], v_ext[t - 1][:, hA, :], pT_[:, 0:512],
                                start=st, stop=sp,
                            )
                            nc.tensor.matmul(
                                ctxB[:], v_ext[t - 1][:, hB, :], pT_[:, 512:1024],
                                start=st, stop=sp,
                            )
                        fill_slot()
                    csA = cxp.tile([65, 512], F32, name="csA", tag="cs")
                    nc.vector.tensor_copy(csA[:], ctxA[:])
                    csB = cxp.tile([65, 512], F32, name="csB", tag="cs")
                    nc.vector.tensor_copy(csB[:], ctxB[:])
                    for i in range(4):
                        pieces.append(piece(csA, sc, hA, i))
                        pieces.append(piece(csB, sc, hB, i))

            # drain remaining filler work
            while work:
                work.popleft()()
            while pieces:
                pieces.popleft()()

    nc.compile()
    return nc


def kernel(Q, Wq, bq, Wk, bk, Wv, bv):
    global _cached_nc, last_result
    Q = np.asarray(Q, dtype=np.float32)
    Wq, Wk, Wv = (np.asarray(w, dtype=np.float32) for w in (Wq, Wk, Wv))
    bq, bk, bv = (np.asarray(b, dtype=np.float32) for b in (bq, bk, bv))
    B = Q.shape[0]
    assert Q.shape == (B, S, D) and B * 4 == N_CORES

    if _cached_nc is None:
        _cached_nc = _build()
    nc = _cached_nc

    # host-side shard prep
    qts = [_round_fp32r(Q[b].T) for b in range(B)]
    wqs = [_round_fp32r(Wq[g * J : (g + 1) * J, :].T) for g in range(4)]
    wks = [_round_fp32r(Wk[g * J : (g + 1) * J, :].T) for g in range(4)]
    wvs = [_round_fp32r(Wv[g * J : (g + 1) * J, :].T) for g in range(4)]

    in_maps = []
    for c in range(N_CORES):
        b, g = c // 4, c % 4
        jsl = slice(g * J, (g + 1) * J)
        in_maps.append(
            {
                "qt": qts[b],
                "wq": wqs[g],
                "wk": wks[g],
                "wv": wvs[g],
                "bq": np.ascontiguousarray(bq[jsl]),
                "bk": np.ascontiguousarray(bk[jsl]),
                "bv": np.ascontiguousarray(bv[jsl]),
            }
        )

    last_result = run_bass_kernel_spmd(nc, in_maps, list(range(N_CORES)))

    full = np.empty((B, S, D), dtype=np.float32)
    for c in range(N_CORES):
        b, g = c // 4, c % 4
        full[b, :, g * J : (g + 1) * J] = last_result.results[c]["out"]
    return full


# revision 19
# speedup vs baseline: 1.0094x; 1.0094x over previous
"""Multi-head self-attention Trainium2 kernel (8 NeuronCores, SPMD).

Problem: B=2, S=2048, D=1024, H=16, Dk=64; torch-style Linear projections
(x @ W.T + b), custom softmax: p = exp(scores/8), attn = p / (sum(p) + 1e-8).

Sharding: 32 (batch, head) pairs over 8 cores -> core c handles batch c//4,
heads [4*(c%4), 4*(c%4)+4). Each core projects only its 256 features of
q/k/v; attention is embarrassingly parallel over (b, h).

Per-core kernel (all matmuls in fp32r: fp32 with 11 mantissa bits, ~3x the
fp32 PE throughput, ~1.2e-4 rounding error):
  - inputs (host-prepped): QT = Q[b].T [1024, 2048]; WqT/WkT/WvT [1024, 256]
    (slices of W.T); biases.
  - qT/kT [256, 2048] = (W slice) @ QT + b   (transposed-space projection;
    bias added as a per-partition scalar during the PSUM->SBUF copy)
  - v     [2048, 256] = QT.T @ WvT           (normal layout; bias folded into
    the final normalize: (p@v)/denom + bv, exact because sum_t p*bv = denom*bv)
  - per head pair: scoresT[t, s] for both heads packed into one PE pass via
    tile_position row groups (0,0)/(64,0), written into one 2-bank PSUM tile
    so a single exp instruction [128,1024] covers both heads (ScalarE is the
    bottleneck engine; its fixed ~0.5us/instruction overhead is halved)
  - ctxT_ext [65, 512-chunk] = [v_h | 1].T @ p accumulated over 16 t-tiles;
    row 64 = softmax denominator
  - finalize: PE-transpose 128-col blocks -> [128, 65]; DVE reciprocal of
    col 64 and scalar_tensor_tensor: out = ctx * (1/denom) + bv

Scheduling: the attention phase is ACT(exp)-bound (~1.3us per t-step); the
PE's spare capacity there is filled with useful work -- the pair-1
projections and the transpose/normalize pipeline -- one or two units per
t-step. This both hides that work entirely and keeps the PE busy enough
that the HAM clock gate never re-throttles it to 1.2GHz (a >3.4us PE idle
anywhere would double every subsequent matmul's duration; observed).

Output per core: [2048, 256] fp32 -> host concatenates features per batch.
"""

import sys

sys.path.insert(0, "/opt/trn_rl_repo")

from collections import deque
from contextlib import ExitStack

import numpy as np

import concourse.bass as bass
import concourse.tile as tile
from concourse import bacc, mybir
from concourse.bass_utils import run_bass_kernel_spmd
from concourse.masks import make_identity

F32 = mybir.dt.float32
F32R = mybir.dt.float32r

S = 2048  # sequence length
D = 1024  # d_model
J = 256  # features per core (4 heads x 64)
NKT = 8  # k-tiles of the d_model contraction
NSC = 4  # s-chunks of 512
NTT = 16  # t-tiles of 128
N_CORES = 8

_cached_nc = None
last_result = None  # BassKernelResults of the most recent run (for test.py)


def _round_fp32r(x: np.ndarray) -> np.ndarray:
    """Round fp32 to fp32r (keep 11 mantissa bits, round to nearest even)."""
    u = np.ascontiguousarray(x, dtype=np.float32).view(np.uint32)
    r = (u.astype(np.uint64) + 0x7FF + ((u >> 12) & 1)) & 0xFFFFF000
    return r.astype(np.uint32).view(np.float32)


def _build():
    nc = bacc.Bacc(None, target_bir_lowering=False)

    qt = nc.dram_tensor("qt", [D, S], F32R, kind="ExternalInput")
    wq = nc.dram_tensor("wq", [D, J], F32R, kind="ExternalInput")
    wk = nc.dram_tensor("wk", [D, J], F32R, kind="ExternalInput")
    wv = nc.dram_tensor("wv", [D, J], F32R, kind="ExternalInput")
    bq = nc.dram_tensor("bq", [J], F32, kind="ExternalInput")
    bk = nc.dram_tensor("bk", [J], F32, kind="ExternalInput")
    bv = nc.dram_tensor("bv", [J], F32, kind="ExternalInput")
    out = nc.dram_tensor("out", [S, J], F32, kind="ExternalOutput")

    with tile.TileContext(nc) as tc, ExitStack() as ctx:
        wts = ctx.enter_context(tc.tile_pool(name="wts", bufs=1))
        qkp = ctx.enter_context(tc.tile_pool(name="qkp", bufs=1))
        vxp = ctx.enter_context(tc.tile_pool(name="vxp", bufs=1))
        bp = ctx.enter_context(tc.tile_pool(name="bp", bufs=1))
        cxp = ctx.enter_context(tc.tile_pool(name="cxp", bufs=6))
        pTp = ctx.enter_context(tc.tile_pool(name="pTp", bufs=3))
        outp = ctx.enter_context(tc.tile_pool(name="outp", bufs=1))
        rp = ctx.enter_context(tc.tile_pool(name="rp", bufs=8))
        qtcp = ctx.enter_context(tc.tile_pool(name="qtc", bufs=4))

        # Weights: 8 k-tiles each of [128, 256], k-major and split across the
        # HWDGE (sync) / SWDGE (gpsimd) queues, interleaved with the first
        # s-chunk of QT below so the k=0 projection matmuls start early
        wq_t = [
            wts.tile([128, J], F32R, name=f"wq{k}", tag=f"wq{k}") for k in range(NKT)
        ]
        wk_t = [
            wts.tile([128, J], F32R, name=f"wk{k}", tag=f"wk{k}") for k in range(NKT)
        ]
        wv_t = [
            wts.tile([128, J], F32R, name=f"wv{k}", tag=f"wv{k}") for k in range(NKT)
        ]
        qtc0 = qtcp.tile([128, NKT, 512], F32R, name="qtc0", tag="qtc")
        for k in range(NKT):
            ksl = slice(k * 128, (k + 1) * 128)
            eng = nc.sync if k % 2 == 0 else nc.scalar
            eng.dma_start(qtc0[:, k, :], qt[ksl, 0:512])
            nc.sync.dma_start(wq_t[k][:], wq[ksl, :])
            nc.scalar.dma_start(wk_t[k][:], wk[ksl, :])
            nc.gpsimd.dma_start(wv_t[k][:], wv[ksl, :])

        # Biases: bq/bk as per-partition scalars [128, 2]; bv broadcast [128, 256]
        bq_t = bp.tile([128, 2], F32, name="bqt")
        nc.sync.dma_start(bq_t[:], bq.rearrange("(m p) -> p m", p=128))
        bk_t = bp.tile([128, 2], F32, name="bkt")
        nc.sync.dma_start(bk_t[:], bk.rearrange("(m p) -> p m", p=128))
        bv_t = bp.tile([128, J], F32, name="bvt")
        bvap = bv[:]
        bv_bcast = bass.AP(
            tensor=bvap.tensor, offset=bvap.offset, ap=[[0, 128], [1, J]]
        )
        nc.sync.dma_start(bv_t[:], bv_bcast)

        ident = bp.tile([128, 128], F32, name="ident")
        make_identity(nc, ident[:])
        scratch = bp.tile([128, 1], F32, name="scratch")

        # Persistent projected tensors
        qT = [qkp.tile([128, S], F32R, name=f"qT{m}", tag=f"qT{m}") for m in range(2)]
        kT = [qkp.tile([128, S], F32R, name=f"kT{m}", tag=f"kT{m}") for m in range(2)]
        v_ext = []
        for t in range(NTT):
            vt = vxp.tile([128, 4, 65], F32R, name=f"vx{t}", tag=f"vx{t}")
            nc.gpsimd.memset(vt[:].bitcast(F32), 1.0)  # ones col [:, h, 64] survives
            v_ext.append(vt)
        # out accumulation tiles, one per 128-row block of the output
        out_tiles = [
            outp.tile([128, J], F32, name=f"ot{b}", tag=f"ot{b}") for b in range(16)
        ]

        def dma_qtc(tile_, sc):
            s0 = sc * 512
            for k in range(NKT):
                eng = nc.sync if k % 2 == 0 else nc.scalar
                eng.dma_start(
                    tile_[:, k, :], qt[k * 128 : (k + 1) * 128, s0 : s0 + 512]
                )

        # ---- Phase 1: kT[0], qT[0] chunk 0, and all of v ----
        phase1_qtc = []
        with tc.tile_pool(name="pps", bufs=1, space="PSUM") as pps:
            for sc in range(NSC):
                s0 = sc * 512
                if sc == 0:
                    qtc = qtc0
                else:
                    qtc = qtcp.tile([128, NKT, 512], F32R, name="qtc", tag="qtc")
                    dma_qtc(qtc, sc)
                # qT[0] is only needed chunk-by-chunk as the pair-0 attention
                # blocks consume it, so chunks 1-3 move to attention filler
                pq = pps.tile([128, 512], F32, name="pq", tag="pq") if sc == 0 else None
                pk = pps.tile([128, 512], F32, name="pk", tag="pk")
                pv = [
                    pps.tile([128, J], F32, name=f"pv{i}", tag=f"pv{i}")
                    for i in range(4)
                ]
                for k in range(NKT):
                    st, sp = (k == 0), (k == NKT - 1)
                    if pq is not None:
                        nc.tensor.matmul(
                            pq[:], wq_t[k][:, 0:128], qtc[:, k, :], start=st, stop=sp
                        )
                    nc.tensor.matmul(
                        pk[:], wk_t[k][:, 0:128], qtc[:, k, :], start=st, stop=sp
                    )
                    for i in range(4):
                        nc.tensor.matmul(
                            pv[i][:],
                            qtc[:, k, i * 128 : (i + 1) * 128],
                            wv_t[k][:],
                            start=st,
                            stop=sp,
                        )
                if pq is not None:
                    nc.vector.tensor_scalar_add(
                        qT[0][:, s0 : s0 + 512], pq[:], bq_t[:, 0:1]
                    )
                nc.vector.tensor_scalar_add(
                    kT[0][:, s0 : s0 + 512], pk[:], bk_t[:, 0:1]
                )
                phase1_qtc.append(qtc)
                for i in range(4):
                    nc.vector.tensor_copy(
                        v_ext[sc * 4 + i][:, :, 0:64],
                        pv[i][:].rearrange("p (h d) -> p h d", h=4),
                    )
                if sc == 0:
                    # pre-load the ACT exp table set during projections so the
                    # first attention exp doesn't stall the pipeline ~2.7us
                    nc.scalar.activation(
                        scratch[:], bq_t[:, 0:1],
                        mybir.ActivationFunctionType.Exp, scale=0.0,
                    )

        # ---- Phase 2: attention, with pair-1 projections and the
        #      transpose/normalize pipeline as PE filler work ----
        with (
            tc.tile_pool(name="aps", bufs=1, space="PSUM") as aps,
            tc.tile_pool(name="p1b", bufs=1, space="PSUM") as p1b,
        ):
            # --- filler: qT[0] chunks 1-3 (read the still-resident phase-1
            #     qtc tiles; must run before proj1b recycles those slots) ---
            q0_state = {}

            def uq0_start(c):
                def f():
                    q0_state[c] = p1b.tile(
                        [128, 512], F32, name="pq0f", tag=f"x{c % 2}"
                    )
                return f

            def uq0_k(c, k):
                def f():
                    st, sp = (k == 0), (k == NKT - 1)
                    nc.tensor.matmul(
                        q0_state[c][:],
                        wq_t[k][:, 0:128],
                        phase1_qtc[c][:, k, :],
                        start=st,
                        stop=sp,
                    )
                return f

            def uq0_copy(c):
                def f():
                    s0 = c * 512
                    nc.vector.tensor_scalar_add(
                        qT[0][:, s0 : s0 + 512], q0_state.pop(c)[:], bq_t[:, 0:1]
                    )
                return f

            # --- filler: pair-1 projection work units ---
            p1_state = {}

            def u_alloc(c):
                def f():
                    px0 = p1b.tile([128, 512], F32, name="px0", tag="x0")
                    px1 = p1b.tile([128, 512], F32, name="px1", tag="x1")
                    p1_state[c] = (phase1_qtc[c], px0, px1)
                return f

            def u_k(c, k):
                def f():
                    qtc2, px0, px1 = p1_state[c]
                    st, sp = (k == 0), (k == NKT - 1)
                    nc.tensor.matmul(
                        px0[:], wq_t[k][:, 128:256], qtc2[:, k, :], start=st, stop=sp
                    )
                    nc.tensor.matmul(
                        px1[:], wk_t[k][:, 128:256], qtc2[:, k, :], start=st, stop=sp
                    )
                return f

            def u_copy(c):
                def f():
                    _, px0, px1 = p1_state.pop(c)
                    s0 = c * 512
                    nc.vector.tensor_scalar_add(
                        qT[1][:, s0 : s0 + 512], px0[:], bq_t[:, 1:2]
                    )
                    nc.vector.tensor_scalar_add(
                        kT[1][:, s0 : s0 + 512], px1[:], bk_t[:, 1:2]
                    )
                return f

            work = deque()
            for c in range(1, NSC):
                work.append(uq0_start(c))
                for k in range(0, NKT, 2):
                    work.append(uq0_k(c, k))
                    work.append(uq0_k(c, k + 1))
                work.append(uq0_copy(c))
            for c in range(NSC):
                work.append(u_alloc(c))
                for k in range(NKT):
                    work.append(u_k(c, k))
                work.append(u_copy(c))

            # --- filler: transpose/normalize pieces ---
            pieces = deque()
            done_cnt = {}
            piece_idx = [0]

            def piece(cs_tile, sc, h, i):
                def f():
                    tagidx = piece_idx[0] % 2
                    piece_idx[0] += 1
                    tp = p1b.tile(
                        [128, 65], F32, name="tp", tag=f"x{tagidx}"
                    )
                    nc.tensor.transpose(
                        tp[:],
                        cs_tile[0:65, i * 128 : (i + 1) * 128],
                        ident[0:65, 0:65],
                    )
                    r = rp.tile([128, 1], F32, name="r", tag="r")
                    nc.vector.reciprocal(r[:], tp[:, 64:65])
                    blk = sc * 4 + i
                    nc.vector.scalar_tensor_tensor(
                        out=out_tiles[blk][:, h * 64 : (h + 1) * 64],
                        in0=tp[:, 0:64],
                        scalar=r[:],
                        in1=bv_t[:, h * 64 : (h + 1) * 64],
                        op0=mybir.AluOpType.mult,
                        op1=mybir.AluOpType.add,
                    )
                    done_cnt[blk] = done_cnt.get(blk, 0) + 1
                    if done_cnt[blk] == 4:
                        nc.sync.dma_start(
                            out[blk * 128 : (blk + 1) * 128, :], out_tiles[blk][:]
                        )
                return f

            def fill_slot():
                # pair-1 projections first (they gate the pair-1 attention
                # blocks), then transpose pieces, which reuse the x0/x1 PSUM
                # banks after the projections retire
                if work:
                    work.popleft()()
                elif pieces:
                    pieces.popleft()()
                    if len(pieces) > 12 and pieces:
                        pieces.popleft()()

            # burst: pair-1 chunk 0 bridges the PSUM pool-transition wait so
            # the PE never idles across the phase boundary (HAM)
            for _ in range(10):
                if work:
                    work.popleft()()

            for pair in range(2):
                for sc in range(NSC):
                    s0 = sc * 512
                    hA, hB = 2 * pair, 2 * pair + 1
                    qTt, kTt = qT[pair], kT[pair]
                    ctxA = aps.tile([65, 512], F32, name="ctxA", tag="ctx", bufs=2)
                    ctxB = aps.tile([65, 512], F32, name="ctxB", tag="ctx", bufs=2)
                    pts = {}
                    for t in range(NTT + 1):
                        if t < NTT:
                            tsl = slice(t * 128, (t + 1) * 128)
                            # both heads' scoresT share one 2-bank tile so a
                            # single exp instruction covers them
                            g = aps.tile(
                                [128, 1024], F32, name="g", tag="grp", bufs=2
                            )
                            nc.tensor.matmul(
                                g[:, 0:512],
                                kTt[0:64, tsl],
                                qTt[0:64, s0 : s0 + 512],
                                start=True,
                                stop=True,
                                tile_position=(0, 0),
                            )
                            nc.tensor.matmul(
                                g[:, 512:1024],
                                kTt[64:128, tsl],
                                qTt[64:128, s0 : s0 + 512],
                                start=True,
                                stop=True,
                                tile_position=(64, 0),
                            )
                            pT_ = pTp.tile([128, 1024], F32R, name="pT_", tag="pT")
                            nc.scalar.activation(
                                pT_[:], g[:],
                                mybir.ActivationFunctionType.Exp, scale=0.125,
                            )
                            pts[t] = pT_
                        if t >= 1:
                            pT_ = pts.pop(t - 1)
                            st, sp = (t - 1 == 0), (t - 1 == NTT - 1)
                            nc.tensor.matmul(
                                ctxA[:], v_ext[t - 1][:, hA, :], pT_[:, 0:512],
                                start=st, stop=sp,
                            )
                            nc.tensor.matmul(
                                ctxB[:], v_ext[t - 1][:, hB, :], pT_[:, 512:1024],
                                start=st, stop=sp,
                            )
                        fill_slot()
                    csA = cxp.tile([65, 512], F32, name="csA", tag="cs")
                    nc.vector.tensor_copy(csA[:], ctxA[:])
                    csB = cxp.tile([65, 512], F32, name="csB", tag="cs")
                    nc.vector.tensor_copy(csB[:], ctxB[:])
                    for i in range(4):
                        pieces.append(piece(csA, sc, hA, i))
                        pieces.append(piece(csB, sc, hB, i))

            # drain remaining filler work
            while work:
                work.popleft()()
            while pieces:
                pieces.popleft()()

    nc.compile()
    return nc


def kernel(Q, Wq, bq, Wk, bk, Wv, bv):
    global _cached_nc, last_result
    Q = np.asarray(Q, dtype=np.float32)
    Wq, Wk, Wv = (np.asarray(w, dtype=np.float32) for w in (Wq, Wk, Wv))
    bq, bk, bv = (np.asarray(b, dtype=np.float32) for b in (bq, bk, bv))
    B = Q.shape[0]
    assert Q.shape == (B, S, D) and B * 4 == N_CORES

    if _cached_nc is None:
        _cached_nc = _build()
    nc = _cached_nc

    # host-side shard prep
    qts = [_round_fp32r(Q[b].T) for b in range(B)]
    wqs = [_round_fp32r(Wq[g * J : (g + 1) * J, :].T) for g in range(4)]
    wks = [_round_fp32r(Wk[g * J : (g + 1) * J, :].T) for g in range(4)]
    wvs = [_round_fp32r(Wv[g * J : (g + 1) * J, :].T) for g in range(4)]

    in_maps = []
    for c in range(N_CORES):
        b, g = c // 4, c % 4
        jsl = slice(g * J, (g + 1) * J)
        in_maps.append(
            {
                "qt": qts[b],
                "wq": wqs[g],
                "wk": wks[g],
                "wv": wvs[g],
                "bq": np.ascontiguousarray(bq[jsl]),
                "bk": np.ascontiguousarray(bk[jsl]),
                "bv": np.ascontiguousarray(bv[jsl]),
            }
        )

    last_result = run_bass_kernel_spmd(nc, in_maps, list(range(N_CORES)))

    full = np.empty((B, S, D), dtype=np.float32)
    for c in range(N_CORES):
        b, g = c // 4, c % 4
        full[b, :, g * J : (g + 1) * J] = last_result.results[c]["out"]
    return full
